# revision 38
# baseline (speedup 1.0000x reference)
"""Trainium2 Bass kernel for nn_DenseAttnProcessor (sparse_attention), v3.

Cross-attention: q = hs@Wq, k/v = ehs@{Wk,Wv}, per-head softmax(qk^T/8 +
col_bias) @ v, @Wo + bo + residual.  B=8 batches -> data-parallel, one batch
per NeuronCore, no collectives.

Key design (see git history for the bf16 v1 at 711us / fp8 v2 at 382us):
  * fp8e4 DoubleRow matmuls for the two 8.6-GFLOP GEMMs (q-projection and the
    stacked probs@[V@Wo] GEMM) and the k/v projections.  Weights host-scaled
    x64 into fp8's sweet spot; descales folded into PSUM-evacuation copies.
  * probs carried as 16*p fp8 (p~1/77 would be fp8-subnormal), M rows as 8*M,
    residual as 128*hs bf16; kernel returns 128*out, host divides by 128.
  * batched softmax normalization: z=exp(scores) packs into the stacked
    [128,10,NQ] layout; per-head denominators via 5 DoubleRow selector
    matmuls -> Dhat [16,NQ]; ONE reciprocal + ONE bf16 copy per chunk; the
    inverse is broadcast back by 10 selector-transpose matmuls and applied by
    10 DVE multiplies.
  * the suppression mask exp(col_bias) is fused into the z pack itself: the
    stacked zs buffer is DMA-prefilled with the host-precomputed fp8 mask
    stack and the pack DMAs run on the gpsimd SWDGE with accum_op=mult
    (zs = eu * z), so no DVE instruction touches the mask at all.  The packs
    also live on the otherwise-idle gpsimd queue, off the sync engine.
  * engine balance: exp on scalar, qT/k/v/M-evacuations on scalar, reciprocal
    + prob-multiplies + residual adds on vector, packs on gpsimd, loads/stores
    on sync.  PE stream per iter interleaves scores(ci), qT(ci+1) DoubleRow
    groups, the lag-1 broadcast matmuls of ci-1, and the lag-1 AV groups so
    the PE never waits on the softmax tail and HAM stays at 8/8.
"""

import sys

for _p in ("/opt/trn_rl_repo",):
    if _p not in sys.path:
        sys.path.insert(0, _p)

import numpy as np
import ml_dtypes

import concourse.mybir as mybir
import concourse.tile as tile
from concourse import bacc
from concourse.bass import ds
from concourse.masks import make_identity

F32 = mybir.dt.float32
BF16 = mybir.dt.bfloat16
F8 = mybir.dt.float8e4
AF = mybir.ActivationFunctionType
DR = mybir.MatmulPerfMode.DoubleRow

B, HW, C, CT, T, H, D = 8, 4096, 1024, 2048, 77, 16, 64
SUPPRESS = 20.0
RT = H * T + 1                # 1233 stacked rows (16*77 head rows + bo row)
NKT = (RT + 127) // 128       # 10 stack tiles
NQ = 512                      # q rows per chunk
NCHUNK = HW // NQ             # 8
BO_TILE, BO_PART = (H * T) // 128, (H * T) % 128   # bo/ones row: tile 9, part 80

NP_F8 = ml_dtypes.float8_e4m3
NP_BF = ml_dtypes.bfloat16
OUT_SCALE = 1.0 / 128.0  # device returns 128*(attn + bo + residual)

# fuse the mask multiply into the pack DMA (SWDGE CCE mult) -- the CoreSim
# supports it but the hardware DMACopy rejects mult, so it stays off and the
# mask multiplies run per stack tile, alternating vector/gpsimd engines.
PACK_MULT = False


def _pack_pieces(h):
    """DMA pieces for packing head h's 77 rows at stacked row 77*h, split at
    128-row tile boundaries: list of (tile_idx, part_base, src_start, nrows)."""
    g = T * h
    pieces = []
    pos = 0
    while pos < T:
        gg = g + pos
        ti, d = gg // 128, gg % 128
        n = min(T - pos, 128 - d)
        pieces.append((ti, d, pos, n))
        pos += n
    return pieces


# stack tile kt is fully packed once head _KT_LAST_HEAD[kt] has been packed
_KT_LAST_HEAD = {kt: min(128 * kt + 127, H * T - 1) // T for kt in range(NKT)}
# Dhat pair t ready after head _KT_LAST_HEAD[2t+1]; emit its matmul two heads
# later so the ~1.5us SWDGE pack latency never stalls the PE stream.  Pairs
# whose slot would land past head 13 are emitted in the iter tail instead.
_DHAT_EMIT = {}
_DHAT_TAIL = []
for _t in range(NKT // 2):
    _eh = _KT_LAST_HEAD[2 * _t + 1] + 3
    if _eh <= 13:
        _DHAT_EMIT.setdefault(_eh, []).append(_t)
    else:
        _DHAT_TAIL.append(_t)
# broadcasts of the lag-1 chunk spread over the first four head pairs
_BC_PLAN = {0: 3, 1: 2, 2: 2, 3: 3}


def build_nc():
    nc = bacc.Bacc("TRN2", target_bir_lowering=False, debug=False)

    hsT8 = nc.dram_tensor("hsT8", [128, C // 128, HW], F8, kind="ExternalInput")
    hsr = nc.dram_tensor("hsr", [128, HW // 128, C], BF16, kind="ExternalInput")
    wq8 = nc.dram_tensor("wq8", [128, C // 128, C], F8, kind="ExternalInput")
    wk8 = nc.dram_tensor("wk8", [128, CT // 128, C], F8, kind="ExternalInput")
    wv8 = nc.dram_tensor("wv8", [128, CT // 128, C], F8, kind="ExternalInput")
    wob = nc.dram_tensor("wob", [128, C // 128, C], BF16, kind="ExternalInput")
    # inner dim padded 77->80: DoubleRow ldweights requires pair-stride % 16 == 0
    ehsT8 = nc.dram_tensor("ehsT8", [128, CT // 128, 80], F8, kind="ExternalInput")
    eus8 = nc.dram_tensor("eus8", [128, NKT, HW], F8, kind="ExternalInput")
    sel8 = nc.dram_tensor("sel8", [128, NKT, H], F8, kind="ExternalInput")
    selT = nc.dram_tensor("selT", [16, NKT, 128], BF16, kind="ExternalInput")
    bo8 = nc.dram_tensor("bo8", [1, C], F8, kind="ExternalInput")
    ones16 = nc.dram_tensor("ones16", [1, NQ], F8, kind="ExternalInput")
    out = nc.dram_tensor("out", [HW, C], F32, kind="ExternalOutput")

    with tile.TileContext(nc) as tc:
        with (
            tc.tile_pool(name="const", bufs=1) as const,
            tc.tile_pool(name="persist", bufs=1) as persist,
        ):
            ident = const.tile([128, 128], BF16)
            make_identity(nc, ident)
            sel_sb = const.tile([128, NKT, H], F8)
            selT_sb = const.tile([16, NKT, 128], BF16)

            kT_sb = persist.tile([128, C // 128, T], BF16)
            m8_sb = persist.tile([128, NKT, C], F8)
            wq_sb = persist.tile([128, C // 128, C], F8)
            # stacked z*eu / prob buffers, parity double-buffered
            zs = [persist.tile([128, NKT, NQ], F8, name=f"zs{b}") for b in range(2)]
            prob = [persist.tile([128, NKT, NQ], F8, name=f"prob{b}") for b in range(2)]
            # garbage partitions beyond the packed rows must be zero: they meet
            # sel=0 / m=0 weights, and fp8 NaN garbage would poison 0*NaN.
            # (zs is fully covered by the eu prefill each chunk; for prob the
            # 16.0 ones-row comes in via DMA and rows 81:128 stay zero.)
            for b in range(2):
                nc.any.memset(prob[b][ds(64, 64), BO_TILE, :], 0.0)
                if not PACK_MULT:
                    nc.any.memset(zs[b][ds(64, 64), BO_TILE, :], 0.0)
                nc.sync.dma_start(
                    prob[b][ds(BO_PART, 1), BO_TILE, :], ones16[:, :]
                )
            nc.any.memset(m8_sb[ds(64, 64), BO_TILE, :], 0.0)
            nc.sync.dma_start(m8_sb[ds(BO_PART, 1), BO_TILE, :], bo8[:, :])

            # ---------------- stage A: k, v, kT, vT, M ----------------
            with (
                tc.tile_pool(name="sa_sb", bufs=1) as sa_sb,
                tc.tile_pool(name="sa_ps", bufs=2, space="PSUM") as sa_ps,
            ):
                # DMA emission order = sync-queue order: the two tensors the
                # first matmuls need come first, the rest overlaps compute.
                ehsT_sb = sa_sb.tile([128, CT // 128, 80], F8)
                nc.sync.dma_start(ehsT_sb, ehsT8[:, :, :])
                # wk/wv land in nh halves so the first kv matmuls start after
                # ~1MB of DMA instead of waiting for the whole 2.1MB tensor
                wk_sb = sa_sb.tile([128, CT // 128, C], F8)
                wv_sb = sa_sb.tile([128, CT // 128, C], F8)
                for half in range(2):
                    nc.sync.dma_start(
                        wk_sb[:, :, ds(512 * half, 512)],
                        wk8[:, :, ds(512 * half, 512)],
                    )
                for half in range(2):
                    nc.sync.dma_start(
                        wv_sb[:, :, ds(512 * half, 512)],
                        wv8[:, :, ds(512 * half, 512)],
                    )
                nc.sync.dma_start(wq_sb, wq8[:, :, :])
                wo_sb = sa_sb.tile([128, C // 128, C], BF16)
                nc.sync.dma_start(wo_sb, wob[:, :, :])
                nc.sync.dma_start(sel_sb, sel8[:, :, :])
                nc.sync.dma_start(selT_sb, selT[:, :, :])

                kv_sb = {}
                for name, wten in (("k", wk_sb), ("v", wv_sb)):
                    kv_ps = sa_ps.tile([T, C], F32, tag="kvps", bufs=1)
                    for nh in range(2):
                        for j in range(CT // 256):
                            nc.tensor.matmul(
                                kv_ps[:, ds(512 * nh, 512)],
                                ehsT_sb[:, ds(2 * j, 2), :T],
                                wten[:, ds(2 * j, 2), ds(512 * nh, 512)],
                                start=(j == 0),
                                stop=(j == CT // 256 - 1),
                                perf_mode=DR,
                            )
                    kvs = sa_sb.tile([T, C], BF16, tag=f"{name}sb", bufs=1)
                    # khat = 64*k -> bf16 k via 1/64 descale on evacuation.
                    # On DVE: the scalar engine must stay clear for iter-0 exps.
                    nc.vector.tensor_scalar_mul(kvs, kv_ps, 1.0 / 64.0)
                    kv_sb[name] = kvs

                vT_sb = sa_sb.tile([128, C // 128, T], BF16)
                for src, dst in ((kv_sb["k"], kT_sb), (kv_sb["v"], vT_sb)):
                    for i in range(C // 128):
                        tp = sa_ps.tile([128, T], BF16, tag="tpa", bufs=2)
                        nc.tensor.transpose(tp, src[:, ds(128 * i, 128)], ident[:T, :T])
                        nc.any.tensor_copy(dst[:, i, :], tp)

                # M_h = v_h @ (64*Wo_h); evacuate at 1/8 -> m8 = 8*M fp8
                for h in range(H):
                    i, po = h // 2, (h % 2) * 64
                    m_ps = sa_ps.tile([T, C], F32, tag="mps", bufs=2)
                    for nh in range(2):
                        nc.tensor.matmul(
                            m_ps[:, ds(512 * nh, 512)],
                            vT_sb[ds(po, 64), i, :],
                            wo_sb[ds(po, 64), i, ds(512 * nh, 512)],
                            start=True,
                            stop=True,
                        )
                    m_stg = sa_sb.tile([T, C], F8, tag="mstg", bufs=2)
                    nc.vector.tensor_scalar_mul(m_stg, m_ps, 1.0 / 8.0)
                    # alternate DGE queues: 32 pack descriptors on gpsimd alone
                    # would delay chunk 0's z packs (and so its Dhat matmuls)
                    eng = nc.gpsimd if h % 2 == 0 else nc.sync
                    for (ti, pb, s0, nr) in _pack_pieces(h):
                        eng.dma_start(
                            m8_sb[ds(pb, nr), ti, :], m_stg[ds(s0, nr), :]
                        )

            # ---------------- stage B: software-pipelined q chunks ----------------
            with (
                tc.tile_pool(name="ld", bufs=2) as ld,
                tc.tile_pool(name="work", bufs=2) as work,
                tc.tile_pool(name="soft", bufs=4) as soft,
                tc.tile_pool(name="spt", bufs=2, space="PSUM") as spt,
                tc.tile_pool(name="gemm", bufs=2, space="PSUM") as gemm,
                tc.tile_pool(name="dps", bufs=1, space="PSUM") as dps,
                tc.tile_pool(name="bcp", bufs=1, space="PSUM") as bcp,
            ):
                st = {}

                def load_hsT(ci):
                    hsT_t = ld.tile([128, C // 128, NQ], F8, tag="hsT", name=f"ht{ci}")
                    nc.sync.dma_start(hsT_t, hsT8[:, :, ds(NQ * ci, NQ)])
                    st.setdefault(ci, {})["hsT"] = hsT_t

                def load_hsr(ci):
                    # bufs=3: consumed by the lag-1 AV two iters after emission
                    hsr_t = ld.tile(
                        [128, NQ // 128, C], BF16, tag="hsr", bufs=3, name=f"hr{ci}"
                    )
                    nc.sync.dma_start(hsr_t, hsr[:, ds(4 * ci, 4), :])
                    st.setdefault(ci, {})["hsr"] = hsr_t

                def prefill_eu(ci):
                    """zs := eu slice; the packs then multiply z in (CCE mult)."""
                    nc.sync.dma_start(
                        zs[ci % 2][:, :, :], eus8[:, :, ds(NQ * ci, NQ)]
                    )

                def load_eu(ci):
                    eu_t = ld.tile([128, NKT, NQ], F8, tag="eu", name=f"eu{ci}")
                    nc.sync.dma_start(eu_t, eus8[:, :, ds(NQ * ci, NQ)])
                    st.setdefault(ci, {})["eu"] = eu_t

                def qt_mms(ci, ij, jlo, jhi):
                    """part of the qT accumulation chain for rows of block ij."""
                    d = st[ci]
                    if "qT" not in d:
                        d["qT"] = work.tile(
                            [128, C // 128, NQ], BF16, tag="qT", name=f"qT{ci}"
                        )
                    if "qps" not in d or d.get("qps_ij") != ij:
                        d["qps"] = gemm.tile(
                            [128, NQ], F32, tag="gps", name=f"qps{ci}_{ij}"
                        )
                        d["qps_ij"] = ij
                    for j in range(jlo, jhi):
                        nc.tensor.matmul(
                            d["qps"],
                            wq_sb[:, ds(2 * j, 2), ds(128 * ij, 128)],
                            d["hsT"][:, ds(2 * j, 2), :],
                            start=(j == 0),
                            stop=(j == C // 256 - 1),
                            perf_mode=DR,
                        )
                    if jhi == C // 256:
                        # qhatT = 512*qT -> bf16 qT/8 (descale + attn scale)
                        nc.scalar.activation(
                            d["qT"][:, ij, :], d["qps"], AF.Copy, scale=1.0 / 512.0
                        )

                def qt_group(ci, ij):
                    qt_mms(ci, ij, 0, C // 256)

                def sm_pair(ci, p):
                    """scores for heads (2p, 2p+1) into one 2-bank psum tile,
                    ONE exp over both, mask-fused packs for both heads."""
                    i = p  # head pair p occupies inner tile i=p (64+64 rows)
                    sT_ps = spt.tile([T, 2, NQ], F32, tag="sT", name=f"sT{ci}_{p}")
                    for sub in range(2):
                        nc.tensor.matmul(
                            sT_ps[:, sub, :],
                            kT_sb[ds(64 * sub, 64), i, :],
                            st[ci]["qT"][ds(64 * sub, 64), i, :],
                            start=True,
                            stop=True,
                        )
                    # deep staging: the ~1-2us pack-DMA completion latency must
                    # never feed back into the exp cadence
                    z8 = soft.tile([T, 2, NQ], F8, tag="z8", bufs=4, name=f"z8_{p}")
                    nc.scalar.activation(z8, sT_ps, AF.Exp)
                    zst = zs[ci % 2]
                    # packs alternate between the sync and gpsimd DGE queues so
                    # neither descriptor generator becomes the softmax pacer
                    for sub in range(2):
                        h = 2 * p + sub
                        eng = nc.gpsimd if h % 2 == 0 else nc.sync
                        for (ti, pb, s0, nr) in _pack_pieces(h):
                            eng.dma_start(
                                zst[ds(pb, nr), ti, :], z8[ds(s0, nr), sub, :]
                            )

                def mul1(ci, kt):
                    """fallback when PACK_MULT is off: zs *= eu per stack tile
                    (alternating engines so neither vector nor gpsimd paces)."""
                    par = ci % 2
                    rows = BO_PART if kt == BO_TILE else 128
                    eng = nc.vector if kt % 2 == 0 else nc.gpsimd
                    eng.tensor_mul(
                        zs[par][ds(0, rows), kt, :],
                        zs[par][ds(0, rows), kt, :],
                        st[ci]["eu"][ds(0, rows), kt, :],
                    )

                def dhat_mm(ci, t):
                    """Dhat [16, NQ] accumulation: pair t of the selector GEMM."""
                    d = st[ci]
                    if "dhat" not in d:
                        d["dhat"] = dps.tile([16, NQ], F32, tag="dh", name=f"dh{ci}")
                    nc.tensor.matmul(
                        d["dhat"],
                        sel_sb[:, ds(2 * t, 2), :],
                        zs[ci % 2][:, ds(2 * t, 2), :],
                        start=(t == 0),
                        stop=(t == NKT // 2 - 1),
                        perf_mode=DR,
                    )

                def norm_head_scalars(ci):
                    """One reciprocal + one bf16 copy for all 16 heads."""
                    dinv = soft.tile([16, NQ], F32, tag="dinv", bufs=2)
                    nc.vector.reciprocal_approx_fast(dinv, st[ci]["dhat"])
                    dinv_bf = soft.tile([16, NQ], BF16, tag="dinvbf", bufs=2)
                    nc.vector.tensor_copy(dinv_bf, dinv)
                    st[ci]["dinv_bf"] = dinv_bf

                def bc_mul2(ci, kt):
                    """prob[kt] = zs[kt] * broadcast(dinv): selector-T matmul + mul."""
                    bc_ps = bcp.tile([128, NQ], F32, tag="bc", name=f"bc{ci}_{kt}")
                    nc.tensor.matmul(
                        bc_ps,
                        selT_sb[:, kt, :],
                        st[ci]["dinv_bf"],
                        start=True,
                        stop=True,
                    )
                    par = ci % 2
                    rows = BO_PART if kt == BO_TILE else 128
                    nc.vector.tensor_mul(
                        prob[par][ds(0, rows), kt, :],
                        zs[par][ds(0, rows), kt, :],
                        bc_ps[ds(0, rows), :],
                    )

                def av_group(ci, g):
                    """output block (qj, nh) = divmod(g, 2) of chunk ci."""
                    qj, nh = divmod(g, 2)
                    q0 = NQ * ci
                    pr = prob[ci % 2]
                    o_ps = gemm.tile([128, 512], F32, tag="gps", name=f"ops{ci}_{g}")
                    for t in range(NKT // 2):
                        nc.tensor.matmul(
                            o_ps,
                            pr[:, ds(2 * t, 2), ds(128 * qj, 128)],
                            m8_sb[:, ds(2 * t, 2), ds(512 * nh, 512)],
                            start=(t == 0),
                            stop=(t == NKT // 2 - 1),
                            perf_mode=DR,
                        )
                    o_sb = work.tile([128, 512], F32, tag="osb", bufs=3, name=f"osb{g}")
                    nc.vector.tensor_add(
                        o_sb, o_ps, st[ci]["hsr"][:, qj, ds(512 * nh, 512)]
                    )
                    nc.sync.dma_start(
                        out[ds(q0 + 128 * qj, 128), ds(512 * nh, 512)], o_sb
                    )

                # -------- prologue --------
                load_hsT(0)
                load_hsT(1)
                load_hsr(0)
                if PACK_MULT:
                    prefill_eu(0)
                else:
                    load_eu(0)
                for ij in range(C // 128):
                    qt_group(0, ij)

                # -------- steady-state iterations --------
                # at iter ci the PE stream carries: scores(ci) pairs,
                # qT(ci+1) groups, bc(ci-1) + AV(ci-1) (both lag-1, fully
                # ready at iter start), and the Dhat(ci) chain (emitted two
                # heads behind the packs that feed it).
                # At iter ci: the pairs section carries scores(ci), qT(ci+1)
                # and 5 of the 8 lag-1 AV(ci-1) groups -- all of whose inputs
                # completed last iter, so the PE stream never waits on this
                # iter's softmax.  The tail normalizes chunk ci (Dhat already
                # accumulated in-stream) and runs its 10 broadcast+multiply
                # steps with the 3 remaining AV groups as PE filler between
                # them, keeping HAM at 8/8 across the iter boundary.
                # qT groups are front-loaded onto pairs 0-5 (doubled on 4-5)
                # so their scalar-engine evacuation copies -- which release the
                # shared gemm psum slots -- complete before the lag-1 AV tail
                # groups need those slots at the iter end.
                _QT_PLAN = {0: [0], 1: [1], 2: [2], 3: [3], 4: [4, 5], 5: [6, 7]}
                for ci in range(NCHUNK):
                    if ci + 2 < NCHUNK:
                        load_hsT(ci + 2)
                    if ci + 1 < NCHUNK:
                        load_hsr(ci + 1)
                    bc_left = list(range(NKT))
                    for p in range(8):  # head pairs
                        sm_pair(ci, p)
                        for h in (2 * p, 2 * p + 1):
                            if not PACK_MULT:
                                for kt in range(NKT):
                                    if _KT_LAST_HEAD[kt] == h:
                                        mul1(ci, kt)
                            for t in _DHAT_EMIT.get(h, []):
                                dhat_mm(ci, t)
                        if ci + 1 < NCHUNK:
                            for ij in _QT_PLAN.get(p, []):
                                qt_group(ci + 1, ij)
                        if ci > 0:
                            if p < 4:  # 10 broadcasts over the first 4 pairs
                                for _ in range(_BC_PLAN[p]):
                                    bc_mul2(ci - 1, bc_left.pop(0))
                            else:  # AV groups 0-3 on pairs 4-7
                                av_group(ci - 1, p - 4)
                    if ci > 0:
                        for g in range(4, 8):
                            av_group(ci - 1, g)
                    for t in _DHAT_TAIL:
                        dhat_mm(ci, t)
                    norm_head_scalars(ci)
                    if ci + 1 < NCHUNK:
                        if PACK_MULT:
                            prefill_eu(ci + 1)
                        else:
                            load_eu(ci + 1)
                    if ci > 1:
                        st.pop(ci - 2, None)

                # -------- epilogue: norm + AV of the last chunk --------
                # (every AV matmul reads ALL stack tiles, so all broadcasts
                # must be emitted before the first AV group)
                ci = NCHUNK - 1
                for kt in range(NKT):
                    bc_mul2(ci, kt)
                for g in range(8):
                    av_group(ci, g)

    nc.compile()
    return nc


_NC_CACHE = {}


def get_nc():
    if "nc" not in _NC_CACHE:
        _NC_CACHE["nc"] = build_nc()
    return _NC_CACHE["nc"]


def _f8(x):
    return np.clip(np.asarray(x, np.float32), -240.0, 240.0).astype(NP_F8)


def _bf(x):
    return np.asarray(x, dtype=NP_BF)


def _tile_rows(a, p=128):
    """[R, N] -> [p, R//p, N] with out[q, j, n] = a[j*p+q, n]."""
    R, N = a.shape
    return np.ascontiguousarray(a.reshape(R // p, p, N).transpose(1, 0, 2))


def make_in_maps(inputs):
    hs = np.asarray(inputs["hidden_states"], dtype=np.float32)
    ehs = np.asarray(inputs["encoder_hidden_states"], dtype=np.float32)
    mask_A = np.asarray(inputs["mask_A"], dtype=np.float32)
    mask_B = np.asarray(inputs["mask_B"], dtype=np.float32)
    Wq = np.asarray(inputs["Wq"], dtype=np.float32)
    Wk = np.asarray(inputs["Wk"], dtype=np.float32)
    Wv = np.asarray(inputs["Wv"], dtype=np.float32)
    Wo = np.asarray(inputs["Wo"], dtype=np.float32)
    bo = np.asarray(inputs["bo"], dtype=np.float32)
    idxA = np.asarray(inputs["token_indices_A"]).astype(np.int64) % T
    idxB = np.asarray(inputs["token_indices_B"]).astype(np.int64) % T

    # suppression as multiplicative mask exp(col_bias)^T [T, HW]
    col_bias = np.zeros((HW, T), np.float32)
    col_bias[:, idxA] = (-SUPPRESS * (1.0 - mask_A))[:, None]
    col_bias[:, idxB] = (-SUPPRESS * (1.0 - mask_B))[:, None]
    eu = np.exp(col_bias.T)  # [T, HW]

    # stacked-layout tensors [128, NKT, *]
    eus = np.zeros((128, NKT, HW), np.float32)
    sel = np.zeros((128, NKT, H), np.float32)
    selTm = np.zeros((16, NKT, 128), np.float32)
    for r in range(H * T):
        kt, p = divmod(r, 128)
        h, t = r // T, r % T
        eus[p, kt, :] = eu[t, :]
        sel[p, kt, h] = 1.0 / 16.0
        selTm[h, kt, p] = 1.0
    eus8_np = _f8(eus)
    sel8_np = _f8(sel)
    selT_np = _bf(selTm)

    wq8_np = _f8(_tile_rows(Wq * 64.0))
    wk8_np = _f8(_tile_rows(Wk * 64.0))
    wv8_np = _f8(_tile_rows(Wv * 64.0))
    wob_np = _bf(_tile_rows(Wo * 64.0))
    bo8_np = _f8(8.0 * bo)[None, :]

    in_maps = []
    for b in range(B):
        hsT = np.ascontiguousarray(hs[b].T)          # [C, HW]
        in_maps.append(
            {
                "hsT8": _f8(_tile_rows(hsT)),
                "hsr": _bf(_tile_rows(hs[b]) * 128.0),
                "wq8": wq8_np,
                "wk8": wk8_np,
                "wv8": wv8_np,
                "wob": wob_np,
                "ehsT8": np.pad(
                    _f8(_tile_rows(ehs[b].T.copy())), ((0, 0), (0, 0), (0, 3))
                ),
                "eus8": eus8_np,
                "sel8": sel8_np,
                "selT": selT_np,
                "bo8": bo8_np,
                "ones16": np.full((1, NQ), 16.0, NP_F8),
            }
        )
    return in_maps


def kernel(**inputs) -> np.ndarray:
    from concourse.bass_utils import run_bass_kernel_spmd

    nc = get_nc()
    in_maps = make_in_maps(inputs)
    res = run_bass_kernel_spmd(nc, in_maps, core_ids=list(range(B)))
    return (
        np.stack([res.results[b]["out"] for b in range(B)]).astype(np.float32)
        * OUT_SCALE
    )


# revision 50
# speedup vs baseline: 1.0112x; 1.0112x over previous
"""Trainium2 Bass kernel for nn_DenseAttnProcessor (sparse_attention), v3.

Cross-attention: q = hs@Wq, k/v = ehs@{Wk,Wv}, per-head softmax(qk^T/8 +
col_bias) @ v, @Wo + bo + residual.  B=8 batches -> data-parallel, one batch
per NeuronCore, no collectives.

Key design (see git history for the bf16 v1 at 711us / fp8 v2 at 382us):
  * fp8e4 DoubleRow matmuls for the two 8.6-GFLOP GEMMs (q-projection and the
    stacked probs@[V@Wo] GEMM) and the k/v projections.  Weights host-scaled
    x64 into fp8's sweet spot; descales folded into PSUM-evacuation copies.
  * probs carried as 16*p fp8 (p~1/77 would be fp8-subnormal), M rows as 8*M,
    residual as 128*hs bf16; kernel returns 128*out, host divides by 128.
  * batched softmax normalization: z=exp(scores) packs into the stacked
    [128,10,NQ] layout; per-head denominators via 5 DoubleRow selector
    matmuls -> Dhat [16,NQ]; ONE reciprocal + ONE bf16 copy per chunk; the
    inverse is broadcast back by 10 selector-transpose matmuls and applied by
    10 DVE multiplies.
  * the suppression mask exp(col_bias) is fused into the z pack itself: the
    stacked zs buffer is DMA-prefilled with the host-precomputed fp8 mask
    stack and the pack DMAs run on the gpsimd SWDGE with accum_op=mult
    (zs = eu * z), so no DVE instruction touches the mask at all.  The packs
    also live on the otherwise-idle gpsimd queue, off the sync engine.
  * engine balance: exp on scalar, qT/k/v/M-evacuations on scalar, reciprocal
    + prob-multiplies + residual adds on vector, packs on gpsimd, loads/stores
    on sync.  PE stream per iter interleaves scores(ci), qT(ci+1) DoubleRow
    groups, the lag-1 broadcast matmuls of ci-1, and the lag-1 AV groups so
    the PE never waits on the softmax tail and HAM stays at 8/8.
"""

import sys

for _p in ("/opt/trn_rl_repo",):
    if _p not in sys.path:
        sys.path.insert(0, _p)

import numpy as np
import ml_dtypes

import concourse.mybir as mybir
import concourse.tile as tile
from concourse import bacc
from concourse.bass import ds
from concourse.masks import make_identity

F32 = mybir.dt.float32
BF16 = mybir.dt.bfloat16
F8 = mybir.dt.float8e4
AF = mybir.ActivationFunctionType
DR = mybir.MatmulPerfMode.DoubleRow

B, HW, C, CT, T, H, D = 8, 4096, 1024, 2048, 77, 16, 64
SUPPRESS = 20.0
RT = H * T + 1                # 1233 stacked rows (16*77 head rows + bo row)
NKT = (RT + 127) // 128       # 10 stack tiles
NQ = 512                      # q rows per chunk
NCHUNK = HW // NQ             # 8
BO_TILE, BO_PART = (H * T) // 128, (H * T) % 128   # bo/ones row: tile 9, part 80

NP_F8 = ml_dtypes.float8_e4m3
NP_BF = ml_dtypes.bfloat16
OUT_SCALE = 1.0 / 128.0  # device returns 128*(attn + bo + residual)

# fuse the mask multiply into the pack DMA (SWDGE CCE mult) -- the CoreSim
# supports it but the hardware DMACopy rejects mult, so it stays off and the
# mask multiplies run per stack tile, alternating vector/gpsimd engines.
PACK_MULT = False


def _pack_pieces(h):
    """DMA pieces for packing head h's 77 rows at stacked row 77*h, split at
    128-row tile boundaries: list of (tile_idx, part_base, src_start, nrows)."""
    g = T * h
    pieces = []
    pos = 0
    while pos < T:
        gg = g + pos
        ti, d = gg // 128, gg % 128
        n = min(T - pos, 128 - d)
        pieces.append((ti, d, pos, n))
        pos += n
    return pieces


# stack tile kt is fully packed once head _KT_LAST_HEAD[kt] has been packed
_KT_LAST_HEAD = {kt: min(128 * kt + 127, H * T - 1) // T for kt in range(NKT)}
# Dhat pair t ready after head _KT_LAST_HEAD[2t+1]; emit its matmul two heads
# later so the ~1.5us SWDGE pack latency never stalls the PE stream.  Pairs
# whose slot would land past head 13 are emitted in the iter tail instead.
_DHAT_EMIT = {}
_DHAT_TAIL = []
for _t in range(NKT // 2):
    _eh = _KT_LAST_HEAD[2 * _t + 1] + 3
    if _eh <= 13:
        _DHAT_EMIT.setdefault(_eh, []).append(_t)
    else:
        _DHAT_TAIL.append(_t)
# broadcasts of the lag-1 chunk spread over the first four head pairs
_BC_PLAN = {0: 3, 1: 2, 2: 2, 3: 3}


def build_nc():
    nc = bacc.Bacc("TRN2", target_bir_lowering=False, debug=False)

    hsT8 = nc.dram_tensor("hsT8", [128, C // 128, HW], F8, kind="ExternalInput")
    hsr = nc.dram_tensor("hsr", [128, HW // 128, C], BF16, kind="ExternalInput")
    wq8 = nc.dram_tensor("wq8", [128, C // 128, C], F8, kind="ExternalInput")
    wk8 = nc.dram_tensor("wk8", [128, CT // 128, C], F8, kind="ExternalInput")
    wv8 = nc.dram_tensor("wv8", [128, CT // 128, C], F8, kind="ExternalInput")
    wob = nc.dram_tensor("wob", [128, C // 128, C], BF16, kind="ExternalInput")
    # inner dim padded 77->80: DoubleRow ldweights requires pair-stride % 16 == 0
    ehsT8 = nc.dram_tensor("ehsT8", [128, CT // 128, 80], F8, kind="ExternalInput")
    # rank-2 suppression bias operands: indicator rows (A\B, B) and the
    # per-query bias vectors, replicated at partitions {0,1} and {32,33} so
    # the two aug matmuls of a head pair use distinct PE row groups
    augk = nc.dram_tensor("augk", [34, T], BF16, kind="ExternalInput")
    supp = nc.dram_tensor("supp", [34, HW], BF16, kind="ExternalInput")
    sel8 = nc.dram_tensor("sel8", [128, NKT, H], F8, kind="ExternalInput")
    selT = nc.dram_tensor("selT", [16, NKT, 128], BF16, kind="ExternalInput")
    bo8 = nc.dram_tensor("bo8", [1, C], F8, kind="ExternalInput")
    ones16 = nc.dram_tensor("ones16", [1, NQ], F8, kind="ExternalInput")
    out = nc.dram_tensor("out", [HW, C], F32, kind="ExternalOutput")

    with tile.TileContext(nc) as tc:
        with (
            tc.tile_pool(name="const", bufs=1) as const,
            tc.tile_pool(name="persist", bufs=1) as persist,
        ):
            ident = const.tile([128, 128], BF16)
            make_identity(nc, ident)
            sel_sb = const.tile([128, NKT, H], F8)
            selT_sb = const.tile([16, NKT, 128], BF16)
            augk_sb = const.tile([34, T], BF16)
            supp_sb = const.tile([34, HW], BF16)

            kT_sb = persist.tile([128, C // 128, T], BF16)
            m8_sb = persist.tile([128, NKT, C], F8)
            wq_sb = persist.tile([128, C // 128, C], F8)
            # stacked z*eu / prob buffers, parity double-buffered
            zs = [persist.tile([128, NKT, NQ], F8, name=f"zs{b}") for b in range(2)]
            prob = [persist.tile([128, NKT, NQ], F8, name=f"prob{b}") for b in range(2)]
            # garbage partitions beyond the packed rows must be zero: they meet
            # sel=0 / m=0 weights, and fp8 NaN garbage would poison 0*NaN.
            # (zs is fully covered by the eu prefill each chunk; for prob the
            # 16.0 ones-row comes in via DMA and rows 81:128 stay zero.)
            for b in range(2):
                nc.any.memset(prob[b][ds(64, 64), BO_TILE, :], 0.0)
                if not PACK_MULT:
                    nc.any.memset(zs[b][ds(64, 64), BO_TILE, :], 0.0)
                nc.sync.dma_start(
                    prob[b][ds(BO_PART, 1), BO_TILE, :], ones16[:, :]
                )
            nc.any.memset(m8_sb[ds(64, 64), BO_TILE, :], 0.0)
            nc.sync.dma_start(m8_sb[ds(BO_PART, 1), BO_TILE, :], bo8[:, :])

            # ---------------- stage A: k, v, kT, vT, M ----------------
            with (
                tc.tile_pool(name="sa_sb", bufs=1) as sa_sb,
                tc.tile_pool(name="sa_ps", bufs=2, space="PSUM") as sa_ps,
            ):
                # DMA emission order = sync-queue order: the two tensors the
                # first matmuls need come first, the rest overlaps compute.
                ehsT_sb = sa_sb.tile([128, CT // 128, 80], F8)
                nc.sync.dma_start(ehsT_sb, ehsT8[:, :, :])
                wk_sb = sa_sb.tile([128, CT // 128, C], F8)
                nc.sync.dma_start(wk_sb, wk8[:, :, :])
                wv_sb = sa_sb.tile([128, CT // 128, C], F8)
                nc.sync.dma_start(wv_sb, wv8[:, :, :])
                nc.sync.dma_start(wq_sb, wq8[:, :, :])
                wo_sb = sa_sb.tile([128, C // 128, C], BF16)
                nc.sync.dma_start(wo_sb, wob[:, :, :])
                nc.sync.dma_start(sel_sb, sel8[:, :, :])
                nc.sync.dma_start(selT_sb, selT[:, :, :])
                nc.sync.dma_start(augk_sb, augk[:, :])
                nc.sync.dma_start(supp_sb, supp[:, :])

                kv_sb = {}
                for name, wten in (("k", wk_sb), ("v", wv_sb)):
                    kv_ps = sa_ps.tile([T, C], F32, tag="kvps", bufs=1)
                    for nh in range(2):
                        for j in range(CT // 256):
                            nc.tensor.matmul(
                                kv_ps[:, ds(512 * nh, 512)],
                                ehsT_sb[:, ds(2 * j, 2), :T],
                                wten[:, ds(2 * j, 2), ds(512 * nh, 512)],
                                start=(j == 0),
                                stop=(j == CT // 256 - 1),
                                perf_mode=DR,
                            )
                    kvs = sa_sb.tile([T, C], BF16, tag=f"{name}sb", bufs=1)
                    # khat = 64*k -> bf16 k via 1/64 descale on evacuation
                    nc.scalar.activation(kvs, kv_ps, AF.Copy, scale=1.0 / 64.0)
                    kv_sb[name] = kvs

                vT_sb = sa_sb.tile([128, C // 128, T], BF16)
                for src, dst in ((kv_sb["k"], kT_sb), (kv_sb["v"], vT_sb)):
                    for i in range(C // 128):
                        tp = sa_ps.tile([128, T], BF16, tag="tpa", bufs=2)
                        nc.tensor.transpose(tp, src[:, ds(128 * i, 128)], ident[:T, :T])
                        nc.any.tensor_copy(dst[:, i, :], tp)

                # M_h = v_h @ (64*Wo_h); evacuate at 1/8 -> m8 = 8*M fp8
                for h in range(H):
                    i, po = h // 2, (h % 2) * 64
                    m_ps = sa_ps.tile([T, C], F32, tag="mps", bufs=2)
                    for nh in range(2):
                        nc.tensor.matmul(
                            m_ps[:, ds(512 * nh, 512)],
                            vT_sb[ds(po, 64), i, :],
                            wo_sb[ds(po, 64), i, ds(512 * nh, 512)],
                            start=True,
                            stop=True,
                        )
                    m_stg = sa_sb.tile([T, C], F8, tag="mstg", bufs=2)
                    nc.scalar.activation(m_stg, m_ps, AF.Copy, scale=1.0 / 8.0)
                    for (ti, pb, s0, nr) in _pack_pieces(h):
                        nc.gpsimd.dma_start(
                            m8_sb[ds(pb, nr), ti, :], m_stg[ds(s0, nr), :]
                        )

            # ---------------- stage B: software-pipelined q chunks ----------------
            with (
                tc.tile_pool(name="ld", bufs=2) as ld,
                tc.tile_pool(name="work", bufs=2) as work,
                tc.tile_pool(name="soft", bufs=4) as soft,
                tc.tile_pool(name="spt", bufs=2, space="PSUM") as spt,
                tc.tile_pool(name="gemm", bufs=2, space="PSUM") as gemm,
                tc.tile_pool(name="dps", bufs=1, space="PSUM") as dps,
                tc.tile_pool(name="bcp", bufs=1, space="PSUM") as bcp,
            ):
                st = {}

                def load_hsT(ci):
                    hsT_t = ld.tile([128, C // 128, NQ], F8, tag="hsT", name=f"ht{ci}")
                    nc.sync.dma_start(hsT_t, hsT8[:, :, ds(NQ * ci, NQ)])
                    st.setdefault(ci, {})["hsT"] = hsT_t

                def load_hsr(ci):
                    # bufs=3: consumed by the lag-1 AV two iters after emission
                    hsr_t = ld.tile(
                        [128, NQ // 128, C], BF16, tag="hsr", bufs=3, name=f"hr{ci}"
                    )
                    nc.sync.dma_start(hsr_t, hsr[:, ds(4 * ci, 4), :])
                    st.setdefault(ci, {})["hsr"] = hsr_t

                def qt_mms(ci, ij, jlo, jhi):
                    """part of the qT accumulation chain for rows of block ij."""
                    d = st[ci]
                    if "qT" not in d:
                        d["qT"] = work.tile(
                            [128, C // 128, NQ], BF16, tag="qT", name=f"qT{ci}"
                        )
                    if "qps" not in d or d.get("qps_ij") != ij:
                        d["qps"] = gemm.tile(
                            [128, NQ], F32, tag="gps", name=f"qps{ci}_{ij}"
                        )
                        d["qps_ij"] = ij
                    for j in range(jlo, jhi):
                        nc.tensor.matmul(
                            d["qps"],
                            wq_sb[:, ds(2 * j, 2), ds(128 * ij, 128)],
                            d["hsT"][:, ds(2 * j, 2), :],
                            start=(j == 0),
                            stop=(j == C // 256 - 1),
                            perf_mode=DR,
                        )
                    if jhi == C // 256:
                        # qhatT = 512*qT -> bf16 qT/8 (descale + attn scale)
                        nc.scalar.activation(
                            d["qT"][:, ij, :], d["qps"], AF.Copy, scale=1.0 / 512.0
                        )

                def qt_group(ci, ij):
                    qt_mms(ci, ij, 0, C // 256)

                def sm_pair(ci, p):
                    """scores for heads (2p, 2p+1) into one 2-bank psum tile,
                    ONE exp over both, packs for both heads.  The suppression
                    bias is a rank-2 term (col_bias = uA x 1[A\\B] + uB x 1[B]),
                    accumulated into the scores psum by two K=2 matmuls whose
                    row groups (0 and 1) let them overlap the K=64 scores MMs
                    and each other -- so the mask costs ~no PE time and no
                    vector/gpsimd work at all, and lands exactly (pre-exp,
                    fp32) instead of via an fp8 multiplier."""
                    i = p  # head pair p occupies inner tile i=p (64+64 rows)
                    q0 = NQ * ci
                    sT_ps = spt.tile([T, 2, NQ], F32, tag="sT", name=f"sT{ci}_{p}")
                    for sub in range(2):
                        nc.tensor.matmul(
                            sT_ps[:, sub, :],
                            kT_sb[ds(64 * sub, 64), i, :],
                            st[ci]["qT"][ds(64 * sub, 64), i, :],
                            start=True,
                            stop=False,
                        )
                    for sub in range(2):
                        nc.tensor.matmul(
                            sT_ps[:, sub, :],
                            augk_sb[ds(32 * sub, 2), :],
                            supp_sb[ds(32 * sub, 2), ds(q0, NQ)],
                            start=False,
                            stop=True,
                        )
                    # deep staging: the ~1-2us pack-DMA completion latency must
                    # never feed back into the exp cadence
                    z8 = soft.tile([T, 2, NQ], F8, tag="z8", bufs=4, name=f"z8_{p}")
                    nc.scalar.activation(z8, sT_ps, AF.Exp)
                    zst = zs[ci % 2]
                    # packs alternate between the sync and gpsimd DGE queues so
                    # neither descriptor generator becomes the softmax pacer
                    for sub in range(2):
                        h = 2 * p + sub
                        eng = nc.gpsimd if h % 2 == 0 else nc.sync
                        for (ti, pb, s0, nr) in _pack_pieces(h):
                            eng.dma_start(
                                zst[ds(pb, nr), ti, :], z8[ds(s0, nr), sub, :]
                            )

                def dhat_mm(ci, t):
                    """Dhat [16, NQ] accumulation: pair t of the selector GEMM."""
                    d = st[ci]
                    if "dhat" not in d:
                        d["dhat"] = dps.tile([16, NQ], F32, tag="dh", name=f"dh{ci}")
                    nc.tensor.matmul(
                        d["dhat"],
                        sel_sb[:, ds(2 * t, 2), :],
                        zs[ci % 2][:, ds(2 * t, 2), :],
                        start=(t == 0),
                        stop=(t == NKT // 2 - 1),
                        perf_mode=DR,
                    )

                def norm_head_scalars(ci):
                    """One reciprocal + one bf16 copy for all 16 heads."""
                    dinv = soft.tile([16, NQ], F32, tag="dinv", bufs=2)
                    nc.vector.reciprocal_approx_fast(dinv, st[ci]["dhat"])
                    dinv_bf = soft.tile([16, NQ], BF16, tag="dinvbf", bufs=2)
                    nc.vector.tensor_copy(dinv_bf, dinv)
                    st[ci]["dinv_bf"] = dinv_bf

                def bc_mul2(ci, kt):
                    """prob[kt] = zs[kt] * broadcast(dinv): selector-T matmul + mul."""
                    bc_ps = bcp.tile([128, NQ], F32, tag="bc", name=f"bc{ci}_{kt}")
                    nc.tensor.matmul(
                        bc_ps,
                        selT_sb[:, kt, :],
                        st[ci]["dinv_bf"],
                        start=True,
                        stop=True,
                    )
                    par = ci % 2
                    rows = BO_PART if kt == BO_TILE else 128
                    nc.vector.tensor_mul(
                        prob[par][ds(0, rows), kt, :],
                        zs[par][ds(0, rows), kt, :],
                        bc_ps[ds(0, rows), :],
                    )

                def av_group(ci, g):
                    """output block (qj, nh) = divmod(g, 2) of chunk ci."""
                    qj, nh = divmod(g, 2)
                    q0 = NQ * ci
                    pr = prob[ci % 2]
                    o_ps = gemm.tile([128, 512], F32, tag="gps", name=f"ops{ci}_{g}")
                    for t in range(NKT // 2):
                        nc.tensor.matmul(
                            o_ps,
                            pr[:, ds(2 * t, 2), ds(128 * qj, 128)],
                            m8_sb[:, ds(2 * t, 2), ds(512 * nh, 512)],
                            start=(t == 0),
                            stop=(t == NKT // 2 - 1),
                            perf_mode=DR,
                        )
                    o_sb = work.tile([128, 512], F32, tag="osb", bufs=3, name=f"osb{g}")
                    nc.vector.tensor_add(
                        o_sb, o_ps, st[ci]["hsr"][:, qj, ds(512 * nh, 512)]
                    )
                    nc.sync.dma_start(
                        out[ds(q0 + 128 * qj, 128), ds(512 * nh, 512)], o_sb
                    )

                # -------- prologue --------
                load_hsT(0)
                load_hsT(1)
                load_hsr(0)
                for ij in range(C // 128):
                    qt_group(0, ij)

                # -------- steady-state iterations --------
                # at iter ci the PE stream carries: scores(ci) pairs,
                # qT(ci+1) groups, bc(ci-1) + AV(ci-1) (both lag-1, fully
                # ready at iter start), and the Dhat(ci) chain (emitted two
                # heads behind the packs that feed it).
                # At iter ci: the pairs section carries scores(ci), qT(ci+1)
                # and 5 of the 8 lag-1 AV(ci-1) groups -- all of whose inputs
                # completed last iter, so the PE stream never waits on this
                # iter's softmax.  The tail normalizes chunk ci (Dhat already
                # accumulated in-stream) and runs its 10 broadcast+multiply
                # steps with the 3 remaining AV groups as PE filler between
                # them, keeping HAM at 8/8 across the iter boundary.
                # qT groups are front-loaded onto pairs 0-5 (doubled on 4-5)
                # so their scalar-engine evacuation copies -- which release the
                # shared gemm psum slots -- complete before the lag-1 AV tail
                # groups need those slots at the iter end.
                _QT_PLAN = {0: [0], 1: [1], 2: [2], 3: [3], 4: [4, 5], 5: [6, 7]}
                for ci in range(NCHUNK):
                    if ci + 2 < NCHUNK:
                        load_hsT(ci + 2)
                    if ci + 1 < NCHUNK:
                        load_hsr(ci + 1)
                    bc_left = list(range(NKT))
                    for p in range(8):  # head pairs
                        sm_pair(ci, p)
                        for h in (2 * p, 2 * p + 1):
                            for t in _DHAT_EMIT.get(h, []):
                                dhat_mm(ci, t)
                        if ci + 1 < NCHUNK:
                            for ij in _QT_PLAN.get(p, []):
                                qt_group(ci + 1, ij)
                        if ci > 0:
                            if p < 4:  # 10 broadcasts over the first 4 pairs
                                for _ in range(_BC_PLAN[p]):
                                    bc_mul2(ci - 1, bc_left.pop(0))
                            else:  # AV groups 0-3 on pairs 4-7
                                av_group(ci - 1, p - 4)
                    if ci > 0:
                        for g in range(4, 8):
                            av_group(ci - 1, g)
                    for t in _DHAT_TAIL:
                        dhat_mm(ci, t)
                    norm_head_scalars(ci)
                    if ci > 1:
                        st.pop(ci - 2, None)

                # -------- epilogue: norm + AV of the last chunk --------
                # (every AV matmul reads ALL stack tiles, so all broadcasts
                # must be emitted before the first AV group)
                ci = NCHUNK - 1
                for kt in range(NKT):
                    bc_mul2(ci, kt)
                for g in range(8):
                    av_group(ci, g)

    nc.compile()
    return nc


_NC_CACHE = {}


def get_nc():
    if "nc" not in _NC_CACHE:
        _NC_CACHE["nc"] = build_nc()
    return _NC_CACHE["nc"]


def _f8(x):
    return np.clip(np.asarray(x, np.float32), -240.0, 240.0).astype(NP_F8)


def _bf(x):
    return np.asarray(x, dtype=NP_BF)


def _tile_rows(a, p=128):
    """[R, N] -> [p, R//p, N] with out[q, j, n] = a[j*p+q, n]."""
    R, N = a.shape
    return np.ascontiguousarray(a.reshape(R // p, p, N).transpose(1, 0, 2))


def make_in_maps(inputs):
    hs = np.asarray(inputs["hidden_states"], dtype=np.float32)
    ehs = np.asarray(inputs["encoder_hidden_states"], dtype=np.float32)
    mask_A = np.asarray(inputs["mask_A"], dtype=np.float32)
    mask_B = np.asarray(inputs["mask_B"], dtype=np.float32)
    Wq = np.asarray(inputs["Wq"], dtype=np.float32)
    Wk = np.asarray(inputs["Wk"], dtype=np.float32)
    Wv = np.asarray(inputs["Wv"], dtype=np.float32)
    Wo = np.asarray(inputs["Wo"], dtype=np.float32)
    bo = np.asarray(inputs["bo"], dtype=np.float32)
    idxA = np.asarray(inputs["token_indices_A"]).astype(np.int64) % T
    idxB = np.asarray(inputs["token_indices_B"]).astype(np.int64) % T

    # rank-2 suppression bias col_bias = uA x 1[A\B] + uB x 1[B] ("set"
    # semantics: B overwrites A on overlap, so A's indicator excludes B).
    # Indicator rows and bias vectors replicated at partitions {0,1}, {32,33}
    # so the pair's two aug matmuls use distinct PE row groups.
    setB = set(idxB.tolist())
    augk_np = np.zeros((34, T), np.float32)
    supp_np = np.zeros((34, HW), np.float32)
    for t in set(idxA.tolist()) - setB:
        augk_np[0, t] = augk_np[32, t] = 1.0
    for t in setB:
        augk_np[1, t] = augk_np[33, t] = 1.0
    for base in (0, 32):
        supp_np[base + 0] = -SUPPRESS * (1.0 - mask_A)
        supp_np[base + 1] = -SUPPRESS * (1.0 - mask_B)
    augk_np = _bf(augk_np)
    supp_np = _bf(supp_np)

    # stacked-layout selector tensors [*, NKT, *]
    sel = np.zeros((128, NKT, H), np.float32)
    selTm = np.zeros((16, NKT, 128), np.float32)
    for r in range(H * T):
        kt, p = divmod(r, 128)
        h = r // T
        sel[p, kt, h] = 1.0 / 16.0
        selTm[h, kt, p] = 1.0
    sel8_np = _f8(sel)
    selT_np = _bf(selTm)

    wq8_np = _f8(_tile_rows(Wq * 64.0))
    wk8_np = _f8(_tile_rows(Wk * 64.0))
    wv8_np = _f8(_tile_rows(Wv * 64.0))
    wob_np = _bf(_tile_rows(Wo * 64.0))
    bo8_np = _f8(8.0 * bo)[None, :]

    in_maps = []
    for b in range(B):
        hsT = np.ascontiguousarray(hs[b].T)          # [C, HW]
        in_maps.append(
            {
                "hsT8": _f8(_tile_rows(hsT)),
                "hsr": _bf(_tile_rows(hs[b]) * 128.0),
                "wq8": wq8_np,
                "wk8": wk8_np,
                "wv8": wv8_np,
                "wob": wob_np,
                "ehsT8": np.pad(
                    _f8(_tile_rows(ehs[b].T.copy())), ((0, 0), (0, 0), (0, 3))
                ),
                "augk": augk_np,
                "supp": supp_np,
                "sel8": sel8_np,
                "selT": selT_np,
                "bo8": bo8_np,
                "ones16": np.full((1, NQ), 16.0, NP_F8),
            }
        )
    return in_maps


def kernel(**inputs) -> np.ndarray:
    from concourse.bass_utils import run_bass_kernel_spmd

    nc = get_nc()
    in_maps = make_in_maps(inputs)
    res = run_bass_kernel_spmd(nc, in_maps, core_ids=list(range(B)))
    return (
        np.stack([res.results[b]["out"] for b in range(B)]).astype(np.float32)
        * OUT_SCALE
    )
